# revision 6
# baseline (speedup 1.0000x reference)
"""Trainium2 Bass kernel for nn_Attention_Critic (gnn_message_passing).

Strategy: data-parallel over the batch (8 cores x 4096), feature-major
layout on chip ([features, batch]), BatchNorm folded into first-layer
weights (stats via one tiny cross-core AllReduce), attention-weight
products folded on host (sel@key^T), attention dots via PE column-reduce
matmuls, softmax computed batch-major, weights transposed back via PE and
broadcast via DMA, weighted sums distributed into the merge/critic
matmuls. bf16 matmuls with fp32 PSUM/stats.
"""
import os
import sys

sys.path.insert(0, "/opt/trn_rl_repo")

import numpy as np
import ml_dtypes
from contextlib import ExitStack

import concourse.bass as bass
import concourse.tile as tile
from concourse import bacc, mybir
from concourse.bass_utils import run_bass_kernel_spmd

NA, B, H = 3, 32768, 128
EPS = 1e-5
NCORES = 8
BS = B // NCORES          # 4096 per core
NT = 512                  # batch tile
ITERS = BS // NT          # 8
SCALE = 1.0 / np.sqrt(H)

bf16 = mybir.dt.bfloat16
f32 = mybir.dt.float32

# first-layer blocks: (name, row offset within agent's 47 ent rows, K)
BLOCKS = [("en", 0, 6), ("oa0", 7, 4), ("oa1", 12, 4), ("g0", 17, 2),
          ("g1", 20, 2), ("g2", 23, 2), ("senc", 26, 20)]
# stat rows (within the agent's 20 stat features) used by each block
BLOCK_STAT = {"en": 0, "oa0": 6, "oa1": 10, "g0": 14, "g1": 16, "g2": 18,
              "senc": 0}
# big packed lhsT weights, 21 x [128,128]
BIGW = (["wsk0", "wsk1", "aval0", "aval1", "mcrit", "cvalw"]
        + [f"m_en{n}" for n in range(NA)] + [f"m_ov0{n}" for n in range(NA)]
        + [f"m_ov1{n}" for n in range(NA)] + [f"cw1a{n}" for n in range(NA)]
        + [f"cw1b{n}" for n in range(NA)])
# bias columns packed [128, 9]
BIASC = ["avb0", "avb1", "mb0", "mb1", "mb2", "cvb", "cb10", "cb11", "cb12"]


def _b16(x):
    return np.asarray(x, np.float32).astype(ml_dtypes.bfloat16)


# ------------------------------------------------------------------
# host-side input prep
# ------------------------------------------------------------------

def _prep_ent_blocks(s, a, lo, hi):
    rows = []
    for n in range(NA):
        sn = s[n, lo:hi].T
        an = a[n, lo:hi].T
        ones = np.ones((1, hi - lo), np.float32)
        rows += [sn[0:4], an[0:2], ones]
        rows += [sn[4:8], ones, sn[8:12], ones]
        rows += [sn[12:14], ones, sn[14:16], ones, sn[16:18], ones]
        rows += [sn[0:4], an[0:2], sn[4:18], ones]
    return np.ascontiguousarray(np.concatenate(rows, 0), dtype=np.float32)


def _prep_l1w(inp):
    out = np.zeros((141, 128), np.float32)
    for n in range(NA):
        o = 47 * n
        out[o + 0:o + 6] = inp["en_W"][n]
        out[o + 6] = inp["en_b"][n]
        out[o + 7:o + 11] = inp["oa_W"][n]
        out[o + 11] = inp["oa_b"][n]
        out[o + 12:o + 16] = inp["oa_W"][n]
        out[o + 16] = inp["oa_b"][n]
        out[o + 17:o + 19] = inp["goal_W"][n]
        out[o + 19] = inp["goal_b"][n]
        out[o + 20:o + 22] = inp["goal_W"][n]
        out[o + 22] = inp["goal_b"][n]
        out[o + 23:o + 25] = inp["goal_W"][n]
        out[o + 25] = inp["goal_b"][n]
        out[o + 26:o + 30] = inp["senc_W"][n][0:4]
        out[o + 32:o + 46] = inp["senc_W"][n][4:18]
        out[o + 46] = inp["senc_b"][n]
    return out


def _prep_bigw(inp):
    w = {}
    w["wsk0"] = inp["asel_W"][0] @ inp["akey_W"][0].T
    w["wsk1"] = inp["asel_W"][1] @ inp["akey_W"][1].T
    w["aval0"] = inp["aval_W"][0]
    w["aval1"] = inp["aval_W"][1]
    w["mcrit"] = inp["ckey_W"][0] @ inp["csel_W"][0].T
    w["cvalw"] = inp["cval_W"][0]
    for n in range(NA):
        w[f"m_en{n}"] = inp["merge_W"][n, 0:128]
        w[f"m_ov0{n}"] = inp["merge_W"][n, 128:256]
        w[f"m_ov1{n}"] = inp["merge_W"][n, 256:384]
        w[f"cw1a{n}"] = inp["cW1"][n, 0:128]
        w[f"cw1b{n}"] = inp["cW1"][n, 128:256]
    return _b16(np.concatenate([w[k] for k in BIGW], 0))      # [21*128, 128]


def _prep_bias(inp):
    cols = [inp["aval_b"][0], inp["aval_b"][1],
            inp["merge_b"][0], inp["merge_b"][1], inp["merge_b"][2],
            inp["cval_b"][0], inp["cb1"][0], inp["cb1"][1], inp["cb1"][2]]
    return np.stack(cols, 1).astype(np.float32)               # [128, 9]


# ------------------------------------------------------------------
# device program
# ------------------------------------------------------------------

_NC_CACHE = {}


def _build_nc():
    nc = bacc.Bacc("TRN2", target_bir_lowering=False, debug=False,
                   num_devices=NCORES)
    entd = nc.dram_tensor("entd", [141, BS], f32, kind="ExternalInput")
    l1wd = nc.dram_tensor("l1wd", [141, 128], f32, kind="ExternalInput")
    bigwd = nc.dram_tensor("bigwd", [21 * 128, 128], bf16, kind="ExternalInput")
    cw2d = nc.dram_tensor("cw2d", [NA * 128, 2], bf16, kind="ExternalInput")
    biasd = nc.dram_tensor("biasd", [128, 9], f32, kind="ExternalInput")
    cb2d = nc.dram_tensor("cb2d", [2, NA], f32, kind="ExternalInput")
    eyed = nc.dram_tensor("eyed", [128, 128], bf16, kind="ExternalInput")
    outd = nc.dram_tensor("outd", [6, BS], f32, kind="ExternalOutput")

    cc_in = nc.dram_tensor("cc_in", [60, 2], f32)
    cc_out = nc.dram_tensor("cc_out", [60, 2], f32, addr_space="Shared")
    rstdd = nc.dram_tensor("rstdd", [60, 1], f32)
    meand = nc.dram_tensor("meand", [60, 1], bf16)
    wscrd = nc.dram_tensor("wscrd", [ITERS, 21, NT], bf16)

    with tile.TileContext(nc) as tc, ExitStack() as ctx:
        wp = ctx.enter_context(tc.tile_pool(name="wp", bufs=1))
        io = ctx.enter_context(tc.tile_pool(name="io", bufs=1))
        wk = ctx.enter_context(tc.tile_pool(name="wk", bufs=2))
        pp = ctx.enter_context(tc.tile_pool(name="pp", bufs=1, space="PSUM"))

        # ---------- weights / constants ----------
        big = {}
        for idx, name in enumerate(BIGW):
            t = wp.tile([128, 128], bf16, name=f"bw_{name}")
            nc.sync.dma_start(t[:], bigwd[128 * idx:128 * (idx + 1), :])
            big[name] = t
        cw2 = []
        for n in range(NA):
            t = wp.tile([128, 2], bf16, name=f"cw2_{n}")
            nc.sync.dma_start(t[:], cw2d[128 * n:128 * (n + 1), :])
            cw2.append(t)
        biast = wp.tile([128, 9], f32)
        nc.sync.dma_start(biast[:], biasd[:, :])
        bcol = {name: biast[:, i:i + 1] for i, name in enumerate(BIASC)}
        cb2t = wp.tile([2, NA], f32)
        nc.sync.dma_start(cb2t[:], cb2d[:, :])
        eyeb = wp.tile([128, 128], bf16)
        nc.sync.dma_start(eyeb[:], eyed[:, :])
        onesb = wp.tile([128, 1], bf16)
        nc.vector.memset(onesb[:], 1.0)
        zbias = wp.tile([128, 1], f32)
        nc.vector.memset(zbias[:], 0.0)

        # ---------- inputs ----------
        # block tiles packed at base partitions {0,32,64} (matmul constraint)
        # group A: en@0(7r), oa0@32(5r), oa1@64(5r); B: g0@0, g1@32, g2@64;
        # C: senc@0(21r)
        GRP = {"en": ("A", 0, 6), "oa0": ("A", 32, 4), "oa1": ("A", 64, 4),
               "g0": ("B", 0, 2), "g1": ("B", 32, 2), "g2": ("B", 64, 2),
               "senc": ("C", 0, 20)}
        GSIZE = {"A": 69, "B": 67, "C": 21}
        ebg = {}
        for n in range(NA):
            for gname in "ABC":
                ebg[(n, gname)] = io.tile([GSIZE[gname], BS], bf16,
                                          name=f"eb{n}{gname}")
        eb = {}
        for n in range(NA):
            o = 47 * n
            for bname, st, K in BLOCKS:
                gname, base, _ = GRP[bname]
                tb = ebg[(n, gname)]
                nc.gpsimd.dma_start(tb[base:base + K + 1, :],
                                    entd[o + st:o + st + K + 1, :])
                eb[(n, bname)] = tb[base:base + K + 1, :]
        sef = io.tile([60, BS], f32, name="sef")
        for n in range(NA):
            o = 47 * n
            nc.sync.dma_start(sef[20 * n:20 * n + 20, :],
                              entd[o + 26:o + 46, :])

        # ---------- stats ----------
        sumx = wp.tile([60, 1], f32)
        sumq = wp.tile([60, 1], f32)
        sq8 = wp.tile([60, 8], f32)
        for c in range(8):
            sqp = pp.tile([60, 512], f32, name="sqp", tag="T5")
            nc.scalar.activation(sqp[:], sef[:, 512 * c:512 * (c + 1)],
                                 mybir.ActivationFunctionType.Square,
                                 accum_out=sq8[:, c:c + 1])
        nc.vector.tensor_reduce(out=sumq[:], in_=sq8[:],
                                op=mybir.AluOpType.add,
                                axis=mybir.AxisListType.X)
        nc.vector.tensor_reduce(out=sumx[:], in_=sef[:],
                                op=mybir.AluOpType.add,
                                axis=mybir.AxisListType.X)
        nc.sync.dma_start(cc_in[:, 0:1], sumx[:])
        nc.sync.dma_start(cc_in[:, 1:2], sumq[:])
        nc.gpsimd.collective_compute(
            "AllReduce", mybir.AluOpType.add,
            replica_groups=[list(range(NCORES))],
            ins=[cc_in[:, :]], outs=[cc_out[:, :]])
        gst = wp.tile([60, 2], f32)
        nc.sync.dma_start(gst[:], cc_out[:, :])
        mean = wp.tile([60, 1], f32)
        nc.vector.tensor_scalar_mul(mean[:], gst[:, 0:1], 1.0 / B)
        ex2 = wp.tile([60, 1], f32)
        nc.vector.tensor_scalar_mul(ex2[:], gst[:, 1:2], 1.0 / B)
        m2 = wp.tile([60, 1], f32)
        nc.vector.tensor_mul(m2[:], mean[:], mean[:])
        var = wp.tile([60, 1], f32)
        nc.vector.tensor_sub(var[:], ex2[:], m2[:])
        epst = wp.tile([60, 1], f32)
        nc.vector.memset(epst[:], EPS)
        std = wp.tile([60, 1], f32)
        nc.scalar.activation(std[:], var[:], mybir.ActivationFunctionType.Sqrt,
                             bias=epst[:])
        rstd = wp.tile([60, 1], f32)
        nc.vector.reciprocal(rstd[:], std[:])
        meanb = wp.tile([60, 1], bf16)
        nc.vector.tensor_copy(meanb[:], mean[:])
        nc.sync.dma_start(rstdd[:, :], rstd[:])
        nc.sync.dma_start(meand[:, :], meanb[:])

        # ---------- fold first-layer weights ----------
        lwg, blkg, rsbg, mbbg = {}, {}, {}, {}
        for n in range(NA):
            for gname in "ABC":
                gsz = GSIZE[gname]
                lwg[(n, gname)] = wp.tile([gsz, 128], f32, name=f"lw{n}{gname}")
                blkg[(n, gname)] = wp.tile([gsz, 128], bf16,
                                           name=f"blk{n}{gname}")
                rsbg[(n, gname)] = wp.tile([gsz, 1], f32, name=f"rsb{n}{gname}")
                mbbg[(n, gname)] = wp.tile([gsz, 1], bf16,
                                           name=f"mbb{n}{gname}")
        blk = {}
        for n in range(NA):
            o = 47 * n
            for bname, st, K in BLOCKS:
                gname, base, _ = GRP[bname]
                so = 20 * n + BLOCK_STAT[bname]
                lw = lwg[(n, gname)]
                bw = blkg[(n, gname)]
                rsb = rsbg[(n, gname)]
                mbb = mbbg[(n, gname)]
                nc.sync.dma_start(lw[base:base + K, :],
                                  l1wd[o + st:o + st + K, :])
                braw = wk.tile([1, 128], f32, name="brawtmp", bufs=3)
                nc.sync.dma_start(braw[:], l1wd[o + st + K:o + st + K + 1, :])
                nc.sync.dma_start(rsb[base:base + K, :], rstdd[so:so + K, :])
                nc.sync.dma_start(mbb[base:base + K, :], meand[so:so + K, :])
                nc.vector.tensor_scalar_mul(bw[base:base + K, :],
                                            lw[base:base + K, :],
                                            rsb[base:base + K, :])
                pb = pp.tile([1, 128], f32, name="pbias", tag="T5")
                nc.tensor.matmul(pb[:], mbb[base:base + K, :],
                                 bw[base:base + K, :], start=True, stop=True)
                brow = wk.tile([1, 128], bf16, name="browtmp", bufs=3)
                nc.vector.tensor_sub(brow[:], braw[:], pb[:])
                nc.sync.dma_start(bw[base + K:base + K + 1, :], brow[:])
                blk[(n, bname)] = bw[base:base + K + 1, :]

        # ---------- main loop ----------
        # PSUM tags (8 banks total):
        #  T0 [2 banks]: l1p0(en,oa0), v1p_a(j0,j1), kmp_a
        #  T1 [2 banks]: l1p1(oa1,g0), v0p, cvp_a
        #  T3 [1 bank]:  l1p3(senc), sk0, v1p_b(j2), kmp_b, cvp_b
        #  T4 [1 bank]:  g1p, sk1, mp, hp
        #  T5 [1 bank]:  g2p, lgp, wfp, clg, cwfp, qp
        # (pbias, sqp from the prefix share T5's tag too)
        LR = mybir.ActivationFunctionType.Lrelu
        OFF = {"en": 0, "oa0": 512, "oa1": 1024, "g0": 1536, "g1": 2048,
               "g2": 2560, "senc": 3072}
        for it in range(ITERS):
            sl = slice(it * NT, (it + 1) * NT)
            sa = [None] * NA
            l1s_t = [None] * NA
            for n in range(NA):
                # ---- L1 matmuls into psum groups
                l1p0 = pp.tile([128, 1024], f32, name="l1p0", tag="T0")
                l1p1 = pp.tile([128, 1024], f32, name="l1p1", tag="T1")
                g1p = pp.tile([128, 512], f32, name="g1p", tag="T4")
                g2p = pp.tile([128, 512], f32, name="g2p", tag="T5")
                l1p3 = pp.tile([128, 512], f32, name="l1p3", tag="T3")
                dests = {"en": (l1p0, 0), "oa0": (l1p0, 512),
                         "oa1": (l1p1, 0), "g0": (l1p1, 512),
                         "g1": (g1p, 0), "g2": (g2p, 0), "senc": (l1p3, 0)}
                for bname, st, K in BLOCKS:
                    pt, off = dests[bname]
                    nc.tensor.matmul(pt[:, off:off + NT], blk[(n, bname)],
                                     eb[(n, bname)][:, sl],
                                     start=True, stop=True)
                l1s = wk.tile([128, 3584], bf16, name="l1s", bufs=3)
                nc.scalar.activation(l1s[:, 0:1024], l1p0[:], LR,
                                     bias=zbias[:], alpha=0.01)
                nc.scalar.activation(l1s[:, 1024:2048], l1p1[:], LR,
                                     bias=zbias[:], alpha=0.01)
                nc.scalar.activation(l1s[:, 2048:2560], g1p[:], LR,
                                     bias=zbias[:], alpha=0.01)
                nc.scalar.activation(l1s[:, 2560:3072], g2p[:], LR,
                                     bias=zbias[:], alpha=0.01)
                nc.scalar.activation(l1s[:, 3072:3584], l1p3[:], LR,
                                     bias=zbias[:], alpha=0.01)
                l1s_t[n] = l1s
                en_ = l1s[:, 0:512]
                encs = [l1s[:, OFF[b]:OFF[b] + 512]
                        for b in ("oa0", "oa1", "g0", "g1", "g2")]
                # ---- selk
                sk0 = pp.tile([128, 512], f32, name="sk0", tag="T3")
                sk1 = pp.tile([128, 512], f32, name="sk1", tag="T4")
                nc.tensor.matmul(sk0[:], big["wsk0"][:], en_,
                                 start=True, stop=True)
                nc.tensor.matmul(sk1[:], big["wsk1"][:], en_,
                                 start=True, stop=True)
                selk = wk.tile([128, 1024], bf16, name="selk")
                nc.vector.tensor_copy(selk[:, 0:512], sk0[:])
                nc.vector.tensor_copy(selk[:, 512:1024], sk1[:])
                # ---- vals
                v0p = pp.tile([128, 1024], f32, name="v0p", tag="T1")
                nc.tensor.matmul(v0p[:, 0:512], big["aval0"][:], encs[0],
                                 start=True, stop=True)
                nc.tensor.matmul(v0p[:, 512:1024], big["aval0"][:], encs[1],
                                 start=True, stop=True)
                vals0 = wk.tile([128, 1024], bf16, name="vals0")
                nc.scalar.activation(vals0[:], v0p[:], LR, bias=bcol["avb0"],
                                     alpha=0.01)
                v1pa = pp.tile([128, 1024], f32, name="v1pa", tag="T0")
                v1pb = pp.tile([128, 512], f32, name="v1pb", tag="T3")
                nc.tensor.matmul(v1pa[:, 0:512], big["aval1"][:], encs[2],
                                 start=True, stop=True)
                nc.tensor.matmul(v1pa[:, 512:1024], big["aval1"][:], encs[3],
                                 start=True, stop=True)
                nc.tensor.matmul(v1pb[:], big["aval1"][:], encs[4],
                                 start=True, stop=True)
                vals1 = wk.tile([128, 1536], bf16, name="vals1")
                nc.scalar.activation(vals1[:, 0:1024], v1pa[:], LR,
                                     bias=bcol["avb1"], alpha=0.01)
                nc.scalar.activation(vals1[:, 1024:1536], v1pb[:], LR,
                                     bias=bcol["avb1"], alpha=0.01)
                # ---- products + column dots -> logits [128, 20] (cols 5t+p)
                lgp = pp.tile([128, 20], f32, name="lgp", tag="T5")
                for p in range(5):
                    sk = selk[:, 0:512] if p < 2 else selk[:, 512:1024]
                    pr = wk.tile([128, 512], bf16, name="pr", bufs=3)
                    nc.gpsimd.tensor_tensor(out=pr[:], in0=sk, in1=encs[p],
                                            op=mybir.AluOpType.mult)
                    for t in range(4):
                        nc.tensor.matmul(lgp[:, 5 * t + p:5 * t + p + 1],
                                         pr[:, 128 * t:128 * (t + 1)],
                                         onesb[:], start=True, stop=True)
                # ---- softmax batch-major
                ebm = wk.tile([128, 20], bf16, name="ebm")
                nc.scalar.activation(ebm[:], lgp[:],
                                     mybir.ActivationFunctionType.Exp,
                                     scale=SCALE)
                den = wk.tile([128, 8], f32, name="den")
                for t in range(4):
                    nc.vector.tensor_reduce(
                        out=den[:, 2 * t:2 * t + 1],
                        in_=ebm[:, 5 * t:5 * t + 2],
                        op=mybir.AluOpType.add, axis=mybir.AxisListType.X)
                    nc.vector.tensor_reduce(
                        out=den[:, 2 * t + 1:2 * t + 2],
                        in_=ebm[:, 5 * t + 2:5 * t + 5],
                        op=mybir.AluOpType.add, axis=mybir.AxisListType.X)
                rec = wk.tile([128, 8], f32, name="rec")
                nc.vector.reciprocal(rec[:], den[:])
                wbm = wk.tile([128, 20], bf16, name="wbm")
                e_oa = ebm[:].rearrange("p (t c) -> p t c", c=5)[:, :, 0:2]
                r_oa = rec[:].rearrange("p (t g) -> p t g", g=2)[:, :, 0:1] \
                    .broadcast_to((128, 4, 2))
                w_oa = wbm[:].rearrange("p (t c) -> p t c", c=5)[:, :, 0:2]
                nc.vector.tensor_tensor(out=w_oa, in0=e_oa, in1=r_oa,
                                        op=mybir.AluOpType.mult)
                e_g = ebm[:].rearrange("p (t c) -> p t c", c=5)[:, :, 2:5]
                r_g = rec[:].rearrange("p (t g) -> p t g", g=2)[:, :, 1:2] \
                    .broadcast_to((128, 4, 3))
                w_g = wbm[:].rearrange("p (t c) -> p t c", c=5)[:, :, 2:5]
                nc.vector.tensor_tensor(out=w_g, in0=e_g, in1=r_g,
                                        op=mybir.AluOpType.mult)
                # ---- transpose w to rows, stash to DRAM
                wfp = pp.tile([5, 512], bf16, name="wfp", tag="T5")
                for t in range(4):
                    nc.tensor.transpose(wfp[:, 128 * t:128 * (t + 1)],
                                        wbm[:, 5 * t:5 * t + 5], eyeb[:])
                wfm = wk.tile([5, 512], bf16, name="wfm")
                nc.vector.tensor_copy(wfm[:], wfp[:])
                nc.sync.dma_start(wscrd[it, 5 * n:5 * n + 5, :], wfm[:])
                # ---- broadcast weights, scale vals, merge
                mp = pp.tile([128, 512], f32, name="mp", tag="T4")
                nc.tensor.matmul(mp[:], big[f"m_en{n}"][:], en_,
                                 start=True, stop=False)
                for p in range(5):
                    wrow = wscrd[it:it + 1, 5 * n + p:5 * n + p + 1, :] \
                        .rearrange("a b n -> (a b) n").broadcast_to((128, NT))
                    wb_ = wk.tile([128, 512], bf16, name=f"wb{p}")
                    nc.sync.dma_start(wb_[:], wrow)
                    vsrc = vals0[:, 512 * p:512 * (p + 1)] if p < 2 else \
                        vals1[:, 512 * (p - 2):512 * (p - 1)]
                    sc = wk.tile([128, 512], bf16, name=f"sc{p}")
                    if p < 2:
                        nc.vector.tensor_tensor(out=sc[:], in0=vsrc, in1=wb_[:],
                                                op=mybir.AluOpType.mult)
                    else:
                        nc.gpsimd.tensor_tensor(out=sc[:], in0=vsrc, in1=wb_[:],
                                                op=mybir.AluOpType.mult)
                    mw = big[f"m_ov0{n}"] if p < 2 else big[f"m_ov1{n}"]
                    nc.tensor.matmul(mp[:], mw[:], sc[:], start=False,
                                     stop=(p == 4))
                sa_n = wk.tile([128, 512], bf16, name="sa", bufs=4)
                nc.scalar.activation(sa_n[:], mp[:], LR, bias=bcol[f"mb{n}"],
                                     alpha=0.01)
                sa[n] = sa_n
            # ---- critic ----
            kmpa = pp.tile([128, 1024], f32, name="kmpa", tag="T0")
            kmpb = pp.tile([128, 512], f32, name="kmpb", tag="T3")
            nc.tensor.matmul(kmpa[:, 0:512], big["mcrit"][:], sa[0][:],
                             start=True, stop=True)
            nc.tensor.matmul(kmpa[:, 512:1024], big["mcrit"][:], sa[1][:],
                             start=True, stop=True)
            nc.tensor.matmul(kmpb[:], big["mcrit"][:], sa[2][:],
                             start=True, stop=True)
            keysM = wk.tile([128, 1536], bf16, name="keysM")
            nc.vector.tensor_copy(keysM[:, 0:1024], kmpa[:])
            nc.vector.tensor_copy(keysM[:, 1024:1536], kmpb[:])
            cvpa = pp.tile([128, 1024], f32, name="cvpa", tag="T1")
            cvpb = pp.tile([128, 512], f32, name="cvpb", tag="T3")
            nc.tensor.matmul(cvpa[:, 0:512], big["cvalw"][:], sa[0][:],
                             start=True, stop=True)
            nc.tensor.matmul(cvpa[:, 512:1024], big["cvalw"][:], sa[1][:],
                             start=True, stop=True)
            nc.tensor.matmul(cvpb[:], big["cvalw"][:], sa[2][:],
                             start=True, stop=True)
            cval = wk.tile([128, 1536], bf16, name="cval")
            nc.scalar.activation(cval[:, 0:1024], cvpa[:], LR,
                                 bias=bcol["cvb"], alpha=0.01)
            nc.scalar.activation(cval[:, 1024:1536], cvpb[:], LR,
                                 bias=bcol["cvb"], alpha=0.01)
            clg = pp.tile([128, 24], f32, name="clg", tag="T5")
            for i in range(NA):
                js = [j for j in range(NA) if j != i]
                se_i = l1s_t[i][:, 3072:3584]
                for k, j in enumerate(js):
                    prc = wk.tile([128, 512], bf16, name="prc", bufs=3)
                    nc.vector.tensor_tensor(
                        out=prc[:], in0=se_i,
                        in1=keysM[:, 512 * j:512 * (j + 1)],
                        op=mybir.AluOpType.mult)
                    c = 2 * i + k
                    for t in range(4):
                        nc.tensor.matmul(clg[:, 6 * t + c:6 * t + c + 1],
                                         prc[:, 128 * t:128 * (t + 1)],
                                         onesb[:], start=True, stop=True)
            cebm = wk.tile([128, 24], bf16, name="cebm")
            nc.scalar.activation(cebm[:], clg[:],
                                 mybir.ActivationFunctionType.Exp, scale=SCALE)
            cden = wk.tile([128, 12], f32, name="cden")
            for t in range(4):
                for i in range(NA):
                    nc.vector.tensor_reduce(
                        out=cden[:, 3 * t + i:3 * t + i + 1],
                        in_=cebm[:, 6 * t + 2 * i:6 * t + 2 * i + 2],
                        op=mybir.AluOpType.add, axis=mybir.AxisListType.X)
            crec = wk.tile([128, 12], f32, name="crec")
            nc.vector.reciprocal(crec[:], cden[:])
            cwbm = wk.tile([128, 24], bf16, name="cwbm")
            c_e = cebm[:].rearrange("p (t i k) -> p t i k", i=3, k=2)
            c_r = crec[:].rearrange("p (t i u) -> p t i u", i=3, u=1) \
                .broadcast_to((128, 4, 3, 2))
            c_w = cwbm[:].rearrange("p (t i k) -> p t i k", i=3, k=2)
            nc.vector.tensor_tensor(out=c_w, in0=c_e, in1=c_r,
                                    op=mybir.AluOpType.mult)
            cwfp = pp.tile([6, 512], bf16, name="cwfp", tag="T5")
            for t in range(4):
                nc.tensor.transpose(cwfp[:, 128 * t:128 * (t + 1)],
                                    cwbm[:, 6 * t:6 * t + 6], eyeb[:])
            cwfm = wk.tile([6, 512], bf16, name="cwfm")
            nc.vector.tensor_copy(cwfm[:], cwfp[:])
            nc.sync.dma_start(wscrd[it, 15:21, :], cwfm[:])
            for i in range(NA):
                js = [j for j in range(NA) if j != i]
                se_i = l1s_t[i][:, 3072:3584]
                hp = pp.tile([128, 512], f32, name="hp", tag="T4")
                nc.tensor.matmul(hp[:], big[f"cw1a{i}"][:], se_i,
                                 start=True, stop=False)
                for k, j in enumerate(js):
                    r = 15 + 2 * i + k
                    wrow = wscrd[it:it + 1, r:r + 1, :] \
                        .rearrange("a b n -> (a b) n").broadcast_to((128, NT))
                    cwb = wk.tile([128, 512], bf16, name=f"cwb{k}")
                    nc.sync.dma_start(cwb[:], wrow)
                    csc = wk.tile([128, 512], bf16, name=f"csc{k}")
                    nc.gpsimd.tensor_tensor(
                        out=csc[:], in0=cval[:, 512 * j:512 * (j + 1)],
                        in1=cwb[:], op=mybir.AluOpType.mult)
                    nc.tensor.matmul(hp[:], big[f"cw1b{i}"][:], csc[:],
                                     start=False, stop=(k == 1))
                h_ = wk.tile([128, 512], bf16, name="h")
                nc.scalar.activation(h_[:], hp[:], LR, bias=bcol[f"cb1{i}"],
                                     alpha=0.01)
                qp = pp.tile([2, 512], f32, name="qp", tag="T5")
                nc.tensor.matmul(qp[:], cw2[i][:], h_[:], start=True, stop=True)
                qs = wk.tile([2, 512], f32, name="qs", bufs=3)
                nc.scalar.activation(qs[:], qp[:],
                                     mybir.ActivationFunctionType.Identity,
                                     bias=cb2t[:, i:i + 1])
                nc.sync.dma_start(outd[2 * i:2 * i + 2, sl], qs[:])

    nc.compile()
    return nc


def _get_nc():
    if "nc" not in _NC_CACHE:
        _NC_CACHE["nc"] = _build_nc()
    return _NC_CACHE["nc"]


# ------------------------------------------------------------------
# public entry point
# ------------------------------------------------------------------

def kernel(s, a, en_W, en_b, oa_W, oa_b, goal_W, goal_b, akey_W, asel_W,
           aval_W, aval_b, merge_W, merge_b, senc_W, senc_b, ckey_W,
           csel_W, cval_W, cval_b, cW1, cb1, cW2, cb2):
    inp = dict(s=s, a=a, en_W=en_W, en_b=en_b, oa_W=oa_W, oa_b=oa_b,
               goal_W=goal_W, goal_b=goal_b, akey_W=akey_W, asel_W=asel_W,
               aval_W=aval_W, aval_b=aval_b, merge_W=merge_W, merge_b=merge_b,
               senc_W=senc_W, senc_b=senc_b, ckey_W=ckey_W, csel_W=csel_W,
               cval_W=cval_W, cval_b=cval_b, cW1=cW1, cb1=cb1, cW2=cW2,
               cb2=cb2)
    inp = {k: np.asarray(v, np.float32) for k, v in inp.items()}
    s_, a_ = inp["s"], inp["a"]

    l1w = _prep_l1w(inp)
    bigw = _prep_bigw(inp)
    cw2 = _b16(np.concatenate([inp["cW2"][n] for n in range(NA)], 0))
    biasc = _prep_bias(inp)
    cb2c = inp["cb2"].T.copy()                     # [2, NA]
    eye = _b16(np.eye(128, dtype=np.float32))

    in_maps = []
    for c in range(NCORES):
        ent = _prep_ent_blocks(s_, a_, c * BS, (c + 1) * BS)
        in_maps.append({"entd": ent, "l1wd": l1w, "bigwd": bigw,
                        "cw2d": cw2, "biasd": biasc, "cb2d": cb2c,
                        "eyed": eye})

    nc = _get_nc()
    trace = os.environ.get("BASS_KERNEL_TRACE") == "1"
    res = run_bass_kernel_spmd(nc, in_maps, core_ids=list(range(NCORES)),
                               trace=trace)
    if trace:
        kernel.last_exec_time_ns = res.exec_time_ns
        kernel.last_results = res

    qfull = np.concatenate([res.results[c]["outd"] for c in range(NCORES)], 1)
    return np.ascontiguousarray(
        np.transpose(qfull.reshape(NA, 2, B), (0, 2, 1))).astype(np.float32)


# revision 9
# speedup vs baseline: 1.2641x; 1.2641x over previous
"""Trainium2 Bass kernel for nn_Attention_Critic (gnn_message_passing).

Strategy: data-parallel over the batch (8 cores x 4096), feature-major
layout on chip ([features, batch]), BatchNorm folded into first-layer
weights (stats via one tiny cross-core AllReduce), attention-weight
products folded on host (sel@key^T), attention dots via PE column-reduce
matmuls, softmax computed batch-major, weights transposed back via PE and
broadcast via DMA, weighted sums distributed into the merge/critic
matmuls. bf16 matmuls with fp32 PSUM/stats.
"""
import os
import sys

sys.path.insert(0, "/opt/trn_rl_repo")

import numpy as np
import ml_dtypes
from contextlib import ExitStack

import concourse.bass as bass
import concourse.tile as tile
from concourse import bacc, mybir
from concourse.bass_utils import run_bass_kernel_spmd

# Pin every activation to the natural_log_exp_and_others table set (covers
# Exp/Ln/Lrelu/Identity/Square/Copy) so the whole kernel needs exactly one
# ACT_TABLE_LOAD instead of thrashing between per-function sets.
_ORIG_GAT = bacc.get_activation_tables


def _pinned_tables(arch):
    t = _ORIG_GAT(arch)
    return {k: (v if k == "natural_log_exp_and_others" else set())
            for k, v in t.items()}


bacc.get_activation_tables = _pinned_tables

NA, B, H = 3, 32768, 128
EPS = 1e-5
NCORES = 8
BS = B // NCORES          # 4096 per core
NT = 512                  # batch tile
ITERS = BS // NT          # 8
SCALE = 1.0 / np.sqrt(H)

bf16 = mybir.dt.bfloat16
f32 = mybir.dt.float32

# first-layer blocks: (name, row offset within agent's 47 ent rows, K)
BLOCKS = [("en", 0, 6), ("oa0", 7, 4), ("oa1", 12, 4), ("g0", 17, 2),
          ("g1", 20, 2), ("g2", 23, 2), ("senc", 26, 20)]
# stat rows (within the agent's 20 stat features) used by each block
BLOCK_STAT = {"en": 0, "oa0": 6, "oa1": 10, "g0": 14, "g1": 16, "g2": 18,
              "senc": 0}
# big packed lhsT weights, 21 x [128,128]
BIGW = (["wsk0", "wsk1", "aval0", "aval1", "mcrit", "cvalw"]
        + [f"m_en{n}" for n in range(NA)] + [f"m_ov0{n}" for n in range(NA)]
        + [f"m_ov1{n}" for n in range(NA)] + [f"cw1a{n}" for n in range(NA)]
        + [f"cw1b{n}" for n in range(NA)])
# bias columns packed [128, 9]
BIASC = ["avb0", "avb1", "mb0", "mb1", "mb2", "cvb", "cb10", "cb11", "cb12"]


def _b16(x):
    return np.asarray(x, np.float32).astype(ml_dtypes.bfloat16)


# ------------------------------------------------------------------
# host-side input prep
# ------------------------------------------------------------------

def _prep_ent_blocks(s, a, lo, hi):
    rows = []
    for n in range(NA):
        sn = s[n, lo:hi].T
        an = a[n, lo:hi].T
        ones = np.ones((1, hi - lo), np.float32)
        rows += [sn[0:4], an[0:2], ones]
        rows += [sn[4:8], ones, sn[8:12], ones]
        rows += [sn[12:14], ones, sn[14:16], ones, sn[16:18], ones]
        rows += [sn[0:4], an[0:2], sn[4:18], ones]
    return np.ascontiguousarray(np.concatenate(rows, 0), dtype=np.float32)


def _prep_l1w(inp):
    out = np.zeros((141, 128), np.float32)
    for n in range(NA):
        o = 47 * n
        out[o + 0:o + 6] = inp["en_W"][n]
        out[o + 6] = inp["en_b"][n]
        out[o + 7:o + 11] = inp["oa_W"][n]
        out[o + 11] = inp["oa_b"][n]
        out[o + 12:o + 16] = inp["oa_W"][n]
        out[o + 16] = inp["oa_b"][n]
        out[o + 17:o + 19] = inp["goal_W"][n]
        out[o + 19] = inp["goal_b"][n]
        out[o + 20:o + 22] = inp["goal_W"][n]
        out[o + 22] = inp["goal_b"][n]
        out[o + 23:o + 25] = inp["goal_W"][n]
        out[o + 25] = inp["goal_b"][n]
        out[o + 26:o + 30] = inp["senc_W"][n][0:4]
        out[o + 32:o + 46] = inp["senc_W"][n][4:18]
        out[o + 46] = inp["senc_b"][n]
    return out


def _prep_bigw(inp):
    w = {}
    w["wsk0"] = inp["asel_W"][0] @ inp["akey_W"][0].T
    w["wsk1"] = inp["asel_W"][1] @ inp["akey_W"][1].T
    w["aval0"] = inp["aval_W"][0]
    w["aval1"] = inp["aval_W"][1]
    w["mcrit"] = inp["ckey_W"][0] @ inp["csel_W"][0].T
    w["cvalw"] = inp["cval_W"][0]
    for n in range(NA):
        w[f"m_en{n}"] = inp["merge_W"][n, 0:128]
        w[f"m_ov0{n}"] = inp["merge_W"][n, 128:256]
        w[f"m_ov1{n}"] = inp["merge_W"][n, 256:384]
        w[f"cw1a{n}"] = inp["cW1"][n, 0:128]
        w[f"cw1b{n}"] = inp["cW1"][n, 128:256]
    return _b16(np.concatenate([w[k] for k in BIGW], 0))      # [21*128, 128]


def _prep_bias(inp):
    cols = [inp["aval_b"][0], inp["aval_b"][1],
            inp["merge_b"][0], inp["merge_b"][1], inp["merge_b"][2],
            inp["cval_b"][0], inp["cb1"][0], inp["cb1"][1], inp["cb1"][2]]
    return np.stack(cols, 1).astype(np.float32)               # [128, 9]


# ------------------------------------------------------------------
# device program
# ------------------------------------------------------------------

_NC_CACHE = {}


def _build_nc():
    nc = bacc.Bacc("TRN2", target_bir_lowering=False, debug=False,
                   num_devices=NCORES)
    entd = nc.dram_tensor("entd", [141, BS], f32, kind="ExternalInput")
    l1wd = nc.dram_tensor("l1wd", [141, 128], f32, kind="ExternalInput")
    bigwd = nc.dram_tensor("bigwd", [21 * 128, 128], bf16, kind="ExternalInput")
    cw2d = nc.dram_tensor("cw2d", [NA * 128, 2], bf16, kind="ExternalInput")
    biasd = nc.dram_tensor("biasd", [128, 9], f32, kind="ExternalInput")
    cb2d = nc.dram_tensor("cb2d", [2, NA], f32, kind="ExternalInput")
    eyed = nc.dram_tensor("eyed", [128, 128], bf16, kind="ExternalInput")
    outd = nc.dram_tensor("outd", [6, BS], f32, kind="ExternalOutput")

    cc_in = nc.dram_tensor("cc_in", [60, 2], f32)
    cc_out = nc.dram_tensor("cc_out", [60, 2], f32, addr_space="Shared")
    rstdd = nc.dram_tensor("rstdd", [60, 1], f32)
    meand = nc.dram_tensor("meand", [60, 1], bf16)
    wscrd = nc.dram_tensor("wscrd", [ITERS, 21, NT], bf16)

    with tile.TileContext(nc) as tc, ExitStack() as ctx:
        wp = ctx.enter_context(tc.tile_pool(name="wp", bufs=1))
        io = ctx.enter_context(tc.tile_pool(name="io", bufs=1))
        wk = ctx.enter_context(tc.tile_pool(name="wk", bufs=2))
        pp = ctx.enter_context(tc.tile_pool(name="pp", bufs=1, space="PSUM"))

        # ---------- weights / constants ----------
        big = {}
        for idx, name in enumerate(BIGW):
            t = wp.tile([128, 128], bf16, name=f"bw_{name}")
            nc.sync.dma_start(t[:], bigwd[128 * idx:128 * (idx + 1), :])
            big[name] = t
        cw2 = []
        for n in range(NA):
            t = wp.tile([128, 2], bf16, name=f"cw2_{n}")
            nc.sync.dma_start(t[:], cw2d[128 * n:128 * (n + 1), :])
            cw2.append(t)
        biast = wp.tile([128, 9], f32)
        nc.sync.dma_start(biast[:], biasd[:, :])
        bcol = {name: biast[:, i:i + 1] for i, name in enumerate(BIASC)}
        cb2t = wp.tile([2, NA], f32)
        nc.sync.dma_start(cb2t[:], cb2d[:, :])
        eyeb = wp.tile([128, 128], bf16)
        nc.sync.dma_start(eyeb[:], eyed[:, :])
        onesb = wp.tile([128, 1], bf16)
        nc.vector.memset(onesb[:], 1.0)
        zbias = wp.tile([128, 1], f32)
        nc.vector.memset(zbias[:], 0.0)

        # ---------- inputs ----------
        # block tiles packed at base partitions {0,32,64} (matmul constraint)
        # group A: en@0(7r), oa0@32(5r), oa1@64(5r); B: g0@0, g1@32, g2@64;
        # C: senc@0(21r)
        GRP = {"en": ("A", 0, 6), "oa0": ("A", 32, 4), "oa1": ("A", 64, 4),
               "g0": ("B", 0, 2), "g1": ("B", 32, 2), "g2": ("B", 64, 2),
               "senc": ("C", 0, 20)}
        GSIZE = {"A": 69, "B": 67, "C": 21}
        ebg = {}
        for n in range(NA):
            for gname in "ABC":
                ebg[(n, gname)] = io.tile([GSIZE[gname], BS], bf16,
                                          name=f"eb{n}{gname}")
        eb = {}
        for n in range(NA):
            o = 47 * n
            for bname, st, K in BLOCKS:
                gname, base, _ = GRP[bname]
                tb = ebg[(n, gname)]
                nc.gpsimd.dma_start(tb[base:base + K + 1, :],
                                    entd[o + st:o + st + K + 1, :])
                eb[(n, bname)] = tb[base:base + K + 1, :]
        sef = io.tile([60, BS], f32, name="sef")
        for n in range(NA):
            o = 47 * n
            nc.sync.dma_start(sef[20 * n:20 * n + 20, :],
                              entd[o + 26:o + 46, :])

        # ---------- stats ----------
        sumx = wp.tile([60, 1], f32)
        sumq = wp.tile([60, 1], f32)
        sq8 = wp.tile([60, 8], f32)
        for c in range(8):
            sqp = pp.tile([60, 512], f32, name="sqp", tag="T5")
            nc.scalar.activation(sqp[:], sef[:, 512 * c:512 * (c + 1)],
                                 mybir.ActivationFunctionType.Square,
                                 accum_out=sq8[:, c:c + 1])
        nc.vector.tensor_reduce(out=sumq[:], in_=sq8[:],
                                op=mybir.AluOpType.add,
                                axis=mybir.AxisListType.X)
        nc.vector.tensor_reduce(out=sumx[:], in_=sef[:],
                                op=mybir.AluOpType.add,
                                axis=mybir.AxisListType.X)
        nc.sync.dma_start(cc_in[:, 0:1], sumx[:])
        nc.sync.dma_start(cc_in[:, 1:2], sumq[:])
        nc.gpsimd.collective_compute(
            "AllReduce", mybir.AluOpType.add,
            replica_groups=[list(range(NCORES))],
            ins=[cc_in[:, :]], outs=[cc_out[:, :]])
        gst = wp.tile([60, 2], f32)
        nc.sync.dma_start(gst[:], cc_out[:, :])
        mean = wp.tile([60, 1], f32)
        nc.vector.tensor_scalar_mul(mean[:], gst[:, 0:1], 1.0 / B)
        ex2 = wp.tile([60, 1], f32)
        nc.vector.tensor_scalar_mul(ex2[:], gst[:, 1:2], 1.0 / B)
        m2 = wp.tile([60, 1], f32)
        nc.vector.tensor_mul(m2[:], mean[:], mean[:])
        var = wp.tile([60, 1], f32)
        nc.vector.tensor_sub(var[:], ex2[:], m2[:])
        epst = wp.tile([60, 1], f32)
        nc.vector.memset(epst[:], EPS)
        lnv = wp.tile([60, 1], f32)
        nc.scalar.activation(lnv[:], var[:], mybir.ActivationFunctionType.Ln,
                             bias=epst[:])
        std = wp.tile([60, 1], f32)
        nc.scalar.activation(std[:], lnv[:], mybir.ActivationFunctionType.Exp,
                             scale=0.5)
        rstd = wp.tile([60, 1], f32)
        nc.vector.reciprocal(rstd[:], std[:])
        meanb = wp.tile([60, 1], bf16)
        nc.vector.tensor_copy(meanb[:], mean[:])
        nc.sync.dma_start(rstdd[:, :], rstd[:])
        nc.sync.dma_start(meand[:, :], meanb[:])

        # ---------- fold first-layer weights ----------
        lwg, blkg, rsbg, mbbg = {}, {}, {}, {}
        for n in range(NA):
            for gname in "ABC":
                gsz = GSIZE[gname]
                lwg[(n, gname)] = wp.tile([gsz, 128], f32, name=f"lw{n}{gname}")
                blkg[(n, gname)] = wp.tile([gsz, 128], bf16,
                                           name=f"blk{n}{gname}")
                rsbg[(n, gname)] = wp.tile([gsz, 1], f32, name=f"rsb{n}{gname}")
                mbbg[(n, gname)] = wp.tile([gsz, 1], bf16,
                                           name=f"mbb{n}{gname}")
        blk = {}
        for n in range(NA):
            o = 47 * n
            for bname, st, K in BLOCKS:
                gname, base, _ = GRP[bname]
                so = 20 * n + BLOCK_STAT[bname]
                lw = lwg[(n, gname)]
                bw = blkg[(n, gname)]
                rsb = rsbg[(n, gname)]
                mbb = mbbg[(n, gname)]
                nc.sync.dma_start(lw[base:base + K, :],
                                  l1wd[o + st:o + st + K, :])
                braw = wk.tile([1, 128], f32, name="brawtmp", bufs=3)
                nc.sync.dma_start(braw[:], l1wd[o + st + K:o + st + K + 1, :])
                nc.sync.dma_start(rsb[base:base + K, :], rstdd[so:so + K, :])
                nc.sync.dma_start(mbb[base:base + K, :], meand[so:so + K, :])
                nc.vector.tensor_scalar_mul(bw[base:base + K, :],
                                            lw[base:base + K, :],
                                            rsb[base:base + K, :])
                pb = pp.tile([1, 128], f32, name="pbias", tag="T5")
                nc.tensor.matmul(pb[:], mbb[base:base + K, :],
                                 bw[base:base + K, :], start=True, stop=True)
                brow = wk.tile([1, 128], bf16, name="browtmp", bufs=3)
                nc.vector.tensor_sub(brow[:], braw[:], pb[:])
                nc.sync.dma_start(bw[base + K:base + K + 1, :], brow[:])
                blk[(n, bname)] = bw[base:base + K + 1, :]

        # ---------- main loop ----------
        # PSUM tags (8 banks total):
        #  T0 [2 banks]: l1p0(en,oa0), v1p_a(j0,j1), kmp_a
        #  T1 [2 banks]: l1p1(oa1,g0), v0p, cvp_a
        #  T3 [1 bank]:  l1p3(senc), sk0, v1p_b(j2), kmp_b, cvp_b
        #  T4 [1 bank]:  g1p, sk1, mp, hp
        #  T5 [1 bank]:  g2p, lgp, wfp, clg, cwfp, qp
        # (pbias, sqp from the prefix share T5's tag too)
        LR = mybir.ActivationFunctionType.Prelu
        OFF = {"en": 0, "oa0": 512, "oa1": 1024, "g0": 1536, "g1": 2048,
               "g2": 2560, "senc": 3072}
        for it in range(ITERS):
            sl = slice(it * NT, (it + 1) * NT)
            sa = [None] * NA
            l1s_t = [None] * NA
            for n in range(NA):
                # ---- L1 matmuls into psum groups
                l1p0 = pp.tile([128, 1024], f32, name="l1p0", tag="T0")
                l1p1 = pp.tile([128, 1024], f32, name="l1p1", tag="T1")
                g1p = pp.tile([128, 512], f32, name="g1p", tag="T4")
                g2p = pp.tile([128, 512], f32, name="g2p", tag="T5")
                l1p3 = pp.tile([128, 512], f32, name="l1p3", tag="T3")
                dests = {"en": (l1p0, 0), "oa0": (l1p0, 512),
                         "oa1": (l1p1, 0), "g0": (l1p1, 512),
                         "g1": (g1p, 0), "g2": (g2p, 0), "senc": (l1p3, 0)}
                for bname, st, K in BLOCKS:
                    pt, off = dests[bname]
                    nc.tensor.matmul(pt[:, off:off + NT], blk[(n, bname)],
                                     eb[(n, bname)][:, sl],
                                     start=True, stop=True)
                l1s = wk.tile([128, 3584], bf16, name="l1s", bufs=3)
                nc.scalar.activation(l1s[:, 0:1024], l1p0[:], LR,
                                     bias=zbias[:], alpha=0.01)
                nc.scalar.activation(l1s[:, 1024:2048], l1p1[:], LR,
                                     bias=zbias[:], alpha=0.01)
                nc.scalar.activation(l1s[:, 2048:2560], g1p[:], LR,
                                     bias=zbias[:], alpha=0.01)
                nc.scalar.activation(l1s[:, 2560:3072], g2p[:], LR,
                                     bias=zbias[:], alpha=0.01)
                nc.scalar.activation(l1s[:, 3072:3584], l1p3[:], LR,
                                     bias=zbias[:], alpha=0.01)
                l1s_t[n] = l1s
                en_ = l1s[:, 0:512]
                encs = [l1s[:, OFF[b]:OFF[b] + 512]
                        for b in ("oa0", "oa1", "g0", "g1", "g2")]
                # ---- selk
                sk0 = pp.tile([128, 512], f32, name="sk0", tag="T3")
                sk1 = pp.tile([128, 512], f32, name="sk1", tag="T4")
                nc.tensor.matmul(sk0[:], big["wsk0"][:], en_,
                                 start=True, stop=True)
                nc.tensor.matmul(sk1[:], big["wsk1"][:], en_,
                                 start=True, stop=True)
                selk = wk.tile([128, 1024], bf16, name="selk")
                nc.vector.tensor_copy(selk[:, 0:512], sk0[:])
                nc.vector.tensor_copy(selk[:, 512:1024], sk1[:])
                # ---- vals
                v0p = pp.tile([128, 1024], f32, name="v0p", tag="T1")
                nc.tensor.matmul(v0p[:, 0:512], big["aval0"][:], encs[0],
                                 start=True, stop=True)
                nc.tensor.matmul(v0p[:, 512:1024], big["aval0"][:], encs[1],
                                 start=True, stop=True)
                vals0 = wk.tile([128, 1024], bf16, name="vals0")
                nc.scalar.activation(vals0[:], v0p[:], LR, bias=bcol["avb0"],
                                     alpha=0.01)
                v1pa = pp.tile([128, 1024], f32, name="v1pa", tag="T0")
                v1pb = pp.tile([128, 512], f32, name="v1pb", tag="T3")
                nc.tensor.matmul(v1pa[:, 0:512], big["aval1"][:], encs[2],
                                 start=True, stop=True)
                nc.tensor.matmul(v1pa[:, 512:1024], big["aval1"][:], encs[3],
                                 start=True, stop=True)
                nc.tensor.matmul(v1pb[:], big["aval1"][:], encs[4],
                                 start=True, stop=True)
                vals1 = wk.tile([128, 1536], bf16, name="vals1")
                nc.scalar.activation(vals1[:, 0:1024], v1pa[:], LR,
                                     bias=bcol["avb1"], alpha=0.01)
                nc.scalar.activation(vals1[:, 1024:1536], v1pb[:], LR,
                                     bias=bcol["avb1"], alpha=0.01)
                # ---- products + column dots -> logits [128, 20] (cols 5t+p)
                lgp = pp.tile([128, 20], f32, name="lgp", tag="T5")
                for p in range(5):
                    sk = selk[:, 0:512] if p < 2 else selk[:, 512:1024]
                    pr = wk.tile([128, 512], bf16, name="pr", bufs=4)
                    peng = nc.vector if p < 2 else nc.gpsimd
                    peng.tensor_tensor(out=pr[:], in0=sk, in1=encs[p],
                                       op=mybir.AluOpType.mult)
                    for t in range(4):
                        nc.tensor.matmul(lgp[:, 5 * t + p:5 * t + p + 1],
                                         pr[:, 128 * t:128 * (t + 1)],
                                         onesb[:], start=True, stop=True)
                # ---- softmax batch-major
                ebm = wk.tile([128, 20], bf16, name="ebm")
                nc.scalar.activation(ebm[:], lgp[:],
                                     mybir.ActivationFunctionType.Exp,
                                     scale=SCALE)
                den = wk.tile([128, 8], f32, name="den")
                nc.vector.tensor_reduce(
                    out=den[:].rearrange("p (t g) -> p t g", g=2)[:, :, 0:1],
                    in_=ebm[:].rearrange("p (t c) -> p t c", c=5)[:, :, 0:2],
                    op=mybir.AluOpType.add, axis=mybir.AxisListType.X)
                nc.vector.tensor_reduce(
                    out=den[:].rearrange("p (t g) -> p t g", g=2)[:, :, 1:2],
                    in_=ebm[:].rearrange("p (t c) -> p t c", c=5)[:, :, 2:5],
                    op=mybir.AluOpType.add, axis=mybir.AxisListType.X)
                rec = wk.tile([128, 8], f32, name="rec")
                nc.vector.reciprocal(rec[:], den[:])
                wbm = wk.tile([128, 20], bf16, name="wbm")
                e_oa = ebm[:].rearrange("p (t c) -> p t c", c=5)[:, :, 0:2]
                r_oa = rec[:].rearrange("p (t g) -> p t g", g=2)[:, :, 0:1] \
                    .broadcast_to((128, 4, 2))
                w_oa = wbm[:].rearrange("p (t c) -> p t c", c=5)[:, :, 0:2]
                nc.vector.tensor_tensor(out=w_oa, in0=e_oa, in1=r_oa,
                                        op=mybir.AluOpType.mult)
                e_g = ebm[:].rearrange("p (t c) -> p t c", c=5)[:, :, 2:5]
                r_g = rec[:].rearrange("p (t g) -> p t g", g=2)[:, :, 1:2] \
                    .broadcast_to((128, 4, 3))
                w_g = wbm[:].rearrange("p (t c) -> p t c", c=5)[:, :, 2:5]
                nc.vector.tensor_tensor(out=w_g, in0=e_g, in1=r_g,
                                        op=mybir.AluOpType.mult)
                # ---- transpose w to rows, stash to DRAM
                wfp = pp.tile([5, 512], bf16, name="wfp", tag="T5")
                for t in range(4):
                    nc.tensor.transpose(wfp[:, 128 * t:128 * (t + 1)],
                                        wbm[:, 5 * t:5 * t + 5], eyeb[:])
                wfm = wk.tile([5, 512], bf16, name="wfm")
                nc.vector.tensor_copy(wfm[:], wfp[:])
                nc.sync.dma_start(wscrd[it, 5 * n:5 * n + 5, :], wfm[:])
                # ---- broadcast weights, scale vals, merge
                mp = pp.tile([128, 512], f32, name="mp", tag="T4")
                nc.tensor.matmul(mp[:], big[f"m_en{n}"][:], en_,
                                 start=True, stop=False)
                for p in range(5):
                    wrow = wscrd[it:it + 1, 5 * n + p:5 * n + p + 1, :] \
                        .rearrange("a b n -> (a b) n").broadcast_to((128, NT))
                    wb_ = wk.tile([128, 512], bf16, name=f"wb{p}")
                    _qeng = [nc.sync, nc.scalar, nc.sync, nc.scalar,
                             nc.sync][p]
                    _qeng.dma_start(wb_[:], wrow)
                    vsrc = vals0[:, 512 * p:512 * (p + 1)] if p < 2 else \
                        vals1[:, 512 * (p - 2):512 * (p - 1)]
                    sc = wk.tile([128, 512], bf16, name=f"sc{p}")
                    nc.vector.tensor_tensor(out=sc[:], in0=vsrc, in1=wb_[:],
                                            op=mybir.AluOpType.mult)
                    mw = big[f"m_ov0{n}"] if p < 2 else big[f"m_ov1{n}"]
                    nc.tensor.matmul(mp[:], mw[:], sc[:], start=False,
                                     stop=(p == 4))
                sa_n = wk.tile([128, 512], bf16, name="sa", bufs=4)
                nc.scalar.activation(sa_n[:], mp[:], LR, bias=bcol[f"mb{n}"],
                                     alpha=0.01)
                sa[n] = sa_n
            # ---- critic ----
            kmpa = pp.tile([128, 1024], f32, name="kmpa", tag="T0")
            kmpb = pp.tile([128, 512], f32, name="kmpb", tag="T3")
            nc.tensor.matmul(kmpa[:, 0:512], big["mcrit"][:], sa[0][:],
                             start=True, stop=True)
            nc.tensor.matmul(kmpa[:, 512:1024], big["mcrit"][:], sa[1][:],
                             start=True, stop=True)
            nc.tensor.matmul(kmpb[:], big["mcrit"][:], sa[2][:],
                             start=True, stop=True)
            keysM = wk.tile([128, 1536], bf16, name="keysM")
            nc.vector.tensor_copy(keysM[:, 0:1024], kmpa[:])
            nc.vector.tensor_copy(keysM[:, 1024:1536], kmpb[:])
            cvpa = pp.tile([128, 1024], f32, name="cvpa", tag="T1")
            cvpb = pp.tile([128, 512], f32, name="cvpb", tag="T3")
            nc.tensor.matmul(cvpa[:, 0:512], big["cvalw"][:], sa[0][:],
                             start=True, stop=True)
            nc.tensor.matmul(cvpa[:, 512:1024], big["cvalw"][:], sa[1][:],
                             start=True, stop=True)
            nc.tensor.matmul(cvpb[:], big["cvalw"][:], sa[2][:],
                             start=True, stop=True)
            cval = wk.tile([128, 1536], bf16, name="cval")
            nc.scalar.activation(cval[:, 0:1024], cvpa[:], LR,
                                 bias=bcol["cvb"], alpha=0.01)
            nc.scalar.activation(cval[:, 1024:1536], cvpb[:], LR,
                                 bias=bcol["cvb"], alpha=0.01)
            clg = pp.tile([128, 24], f32, name="clg", tag="T5")
            for i in range(NA):
                js = [j for j in range(NA) if j != i]
                se_i = l1s_t[i][:, 3072:3584]
                for k, j in enumerate(js):
                    prc = wk.tile([128, 512], bf16, name="prc", bufs=3)
                    nc.vector.tensor_tensor(
                        out=prc[:], in0=se_i,
                        in1=keysM[:, 512 * j:512 * (j + 1)],
                        op=mybir.AluOpType.mult)
                    c = 2 * i + k
                    for t in range(4):
                        nc.tensor.matmul(clg[:, 6 * t + c:6 * t + c + 1],
                                         prc[:, 128 * t:128 * (t + 1)],
                                         onesb[:], start=True, stop=True)
            cebm = wk.tile([128, 24], bf16, name="cebm")
            nc.scalar.activation(cebm[:], clg[:],
                                 mybir.ActivationFunctionType.Exp, scale=SCALE)
            cden = wk.tile([128, 12], f32, name="cden")
            nc.vector.tensor_reduce(
                out=cden[:].rearrange("p (t i) -> p t i", i=3)
                    .rearrange("p t i -> p t i ()"),
                in_=cebm[:].rearrange("p (t i k) -> p t i k", i=3, k=2),
                op=mybir.AluOpType.add, axis=mybir.AxisListType.X)
            crec = wk.tile([128, 12], f32, name="crec")
            nc.vector.reciprocal(crec[:], cden[:])
            cwbm = wk.tile([128, 24], bf16, name="cwbm")
            c_e = cebm[:].rearrange("p (t i k) -> p t i k", i=3, k=2)
            c_r = crec[:].rearrange("p (t i u) -> p t i u", i=3, u=1) \
                .broadcast_to((128, 4, 3, 2))
            c_w = cwbm[:].rearrange("p (t i k) -> p t i k", i=3, k=2)
            nc.vector.tensor_tensor(out=c_w, in0=c_e, in1=c_r,
                                    op=mybir.AluOpType.mult)
            cwfp = pp.tile([6, 512], bf16, name="cwfp", tag="T5")
            for t in range(4):
                nc.tensor.transpose(cwfp[:, 128 * t:128 * (t + 1)],
                                    cwbm[:, 6 * t:6 * t + 6], eyeb[:])
            cwfm = wk.tile([6, 512], bf16, name="cwfm")
            nc.vector.tensor_copy(cwfm[:], cwfp[:])
            nc.sync.dma_start(wscrd[it, 15:21, :], cwfm[:])
            for i in range(NA):
                js = [j for j in range(NA) if j != i]
                se_i = l1s_t[i][:, 3072:3584]
                hp = pp.tile([128, 512], f32, name="hp", tag="T4")
                nc.tensor.matmul(hp[:], big[f"cw1a{i}"][:], se_i,
                                 start=True, stop=False)
                for k, j in enumerate(js):
                    r = 15 + 2 * i + k
                    wrow = wscrd[it:it + 1, r:r + 1, :] \
                        .rearrange("a b n -> (a b) n").broadcast_to((128, NT))
                    cwb = wk.tile([128, 512], bf16, name=f"cwb{k}")
                    _qeng = [nc.scalar, nc.sync][k]
                    _qeng.dma_start(cwb[:], wrow)
                    csc = wk.tile([128, 512], bf16, name=f"csc{k}")
                    ceng = nc.gpsimd if k == 0 else nc.vector
                    ceng.tensor_tensor(
                        out=csc[:], in0=cval[:, 512 * j:512 * (j + 1)],
                        in1=cwb[:], op=mybir.AluOpType.mult)
                    nc.tensor.matmul(hp[:], big[f"cw1b{i}"][:], csc[:],
                                     start=False, stop=(k == 1))
                h_ = wk.tile([128, 512], bf16, name="h")
                nc.scalar.activation(h_[:], hp[:], LR, bias=bcol[f"cb1{i}"],
                                     alpha=0.01)
                qp = pp.tile([2, 512], f32, name="qp", tag="T5")
                nc.tensor.matmul(qp[:], cw2[i][:], h_[:], start=True, stop=True)
                qs = wk.tile([2, 512], f32, name="qs", bufs=3)
                nc.scalar.activation(qs[:], qp[:],
                                     mybir.ActivationFunctionType.Identity,
                                     bias=cb2t[:, i:i + 1])
                nc.scalar.dma_start(outd[2 * i:2 * i + 2, sl], qs[:])

    nc.compile()
    return nc


def _get_nc():
    if "nc" not in _NC_CACHE:
        _NC_CACHE["nc"] = _build_nc()
    return _NC_CACHE["nc"]


# ------------------------------------------------------------------
# public entry point
# ------------------------------------------------------------------

def kernel(s, a, en_W, en_b, oa_W, oa_b, goal_W, goal_b, akey_W, asel_W,
           aval_W, aval_b, merge_W, merge_b, senc_W, senc_b, ckey_W,
           csel_W, cval_W, cval_b, cW1, cb1, cW2, cb2):
    inp = dict(s=s, a=a, en_W=en_W, en_b=en_b, oa_W=oa_W, oa_b=oa_b,
               goal_W=goal_W, goal_b=goal_b, akey_W=akey_W, asel_W=asel_W,
               aval_W=aval_W, aval_b=aval_b, merge_W=merge_W, merge_b=merge_b,
               senc_W=senc_W, senc_b=senc_b, ckey_W=ckey_W, csel_W=csel_W,
               cval_W=cval_W, cval_b=cval_b, cW1=cW1, cb1=cb1, cW2=cW2,
               cb2=cb2)
    inp = {k: np.asarray(v, np.float32) for k, v in inp.items()}
    s_, a_ = inp["s"], inp["a"]

    l1w = _prep_l1w(inp)
    bigw = _prep_bigw(inp)
    cw2 = _b16(np.concatenate([inp["cW2"][n] for n in range(NA)], 0))
    biasc = _prep_bias(inp)
    cb2c = inp["cb2"].T.copy()                     # [2, NA]
    eye = _b16(np.eye(128, dtype=np.float32))

    in_maps = []
    for c in range(NCORES):
        ent = _prep_ent_blocks(s_, a_, c * BS, (c + 1) * BS)
        in_maps.append({"entd": ent, "l1wd": l1w, "bigwd": bigw,
                        "cw2d": cw2, "biasd": biasc, "cb2d": cb2c,
                        "eyed": eye})

    nc = _get_nc()
    trace = os.environ.get("BASS_KERNEL_TRACE") == "1"
    res = run_bass_kernel_spmd(nc, in_maps, core_ids=list(range(NCORES)),
                               trace=trace)
    if trace:
        kernel.last_exec_time_ns = res.exec_time_ns
        kernel.last_results = res

    qfull = np.concatenate([res.results[c]["outd"] for c in range(NCORES)], 1)
    return np.ascontiguousarray(
        np.transpose(qfull.reshape(NA, 2, B), (0, 2, 1))).astype(np.float32)
